# revision 33
# baseline (speedup 1.0000x reference)
"""BEV feature extractor (scatter-max -> 1x1 conv -> BN(train) -> ReLU) on 8 TRN2 cores.

Sharding: data-parallel over (batch, y-strip) -> 8 shards, BN stats all-reduced.

bf16 data path (tolerance 2e-2; bf16 rounding ~4e-3 and BN partially cancels it):

  1. Host packs each shard partition-major: region tensors r0_r [128, RS+1, C]
     hold root features of slot s at [row, s-lo, :] (row = cell's row in the
     slot's 128; last column = dump rows). Colliding cells' points (root +
     extras) are packed into exf fold batches, one batch row per partition,
     balanced so each (region, partition) needs at most NB batches.
  2. Device folds: DVE max-chains over exf slices -> gtf [128, NB, C]; one
     multi-index indirect scatter per region writes the folded rows back into
     r0_r in place (unused rows land on per-partition dump rows). Then the
     region slab DMAs into the SBUF-resident V tile v_all [128, NS, C+1]
     (ones column fused for the sv reduction).
  3. PE accumulates Sigma = sum_s V_s^T [V_s | 1] (bf16). BN stats project to
     per-channel sums BEFORE the collective: ex2_o = w_o^T Sigma w_o,
     mn_o = w_o^T sv, so the AllReduce payload is [128, 2*OCH] (2 KB).
  4. Phase C per quad (4 slots = 1024 cells): GT = V_s^T Sel_s (Sel is a
     host-built bf16 0/1 matrix streamed from DRAM), conv rhs is the bf16 GT
     quad, conv weights are pre-scaled by the BN 'a' (folded on device), so
     the epilogue is relu(x + b) - a single pass alternating ACT / DVE.
     Output is stored bf16 ([O, cells]) and upcast on host.
  5. The collective overlaps with GT run-ahead for the first RA quads.
"""

import math
from dataclasses import dataclass

import ml_dtypes
import numpy as np

import concourse.bass as bass
import concourse.tile as tile
from concourse import bacc, mybir
from concourse.bass_utils import run_bass_kernel_spmd

F32 = mybir.dt.float32
BF16 = mybir.dt.bfloat16
I32 = mybir.dt.int32
BF = ml_dtypes.bfloat16


@dataclass(frozen=True)
class Geo:
    B: int = 2
    H: int = 400
    W: int = 400
    C: int = 128            # input channels (= partition count)
    O: int = 256            # output channels (multiple of 128)
    NSTRIP: int = 4         # y-strips per batch; B*NSTRIP = 8 cores
    SLOT_BLKS: int = 2      # 128-cell blocks packed per 128-row slot
    NREG: int = 4           # independent fold/load regions
    SLICES: tuple = (7, 3, 2, 2, 2)      # points (root+extras) per fold batch
    QUAD: int = 4           # slots per phase-C tile
    RA: int = 16            # run-ahead quads across the collective
    EPS: float = 1e-5

    @property
    def ystrip(self):
        return self.H // self.NSTRIP

    @property
    def cells(self):
        return self.ystrip * self.W

    @property
    def ncores(self):
        return self.B * self.NSTRIP

    @property
    def slot_cells(self):
        return 128 * self.SLOT_BLKS

    @property
    def nslot(self):
        return math.ceil(self.cells / self.slot_cells)

    @property
    def NB(self):
        return len(self.SLICES)

    @property
    def npair(self):                 # exf slices per region
        return sum(self.SLICES)

    @property
    def nquad(self):
        return math.ceil(self.nslot / self.QUAD)

    @property
    def ncell_total(self):
        return self.B * self.H * self.W

    @property
    def reg_bounds(self):
        rs = math.ceil(self.nslot / self.NREG)
        out = []
        for reg in range(self.NREG):
            lo = min(reg * rs, self.nslot)
            hi = self.nslot if reg == self.NREG - 1 else min((reg + 1) * rs, self.nslot)
            out.append((lo, hi))
        return out


GEO = Geo()


# --------------------------------------------------------------------------
# host-side shard prep
# --------------------------------------------------------------------------

def prep_shard(g: Geo, feats: np.ndarray, cell: np.ndarray) -> dict:
    """feats [n, C] f32, cell [n] int in [0, g.cells)."""
    C, SC, NS, NB = g.C, g.slot_cells, g.nslot, g.NB
    slices = g.SLICES
    pair_base = np.cumsum((0,) + slices[:-1])

    order = np.argsort(cell, kind="stable")
    cell_s = cell[order]
    feats_s = feats[order].astype(BF)
    uniq, seg_start, counts = np.unique(
        cell_s, return_index=True, return_counts=True
    )
    slot_of = uniq // SC
    jloc = uniq % SC

    exi = np.zeros((128, NB * g.NREG), np.int32)
    exf = np.zeros((128, g.npair * g.NREG, C), BF)
    sel = np.zeros((128, NS, SC), BF)
    row_of = np.full(len(uniq), -1, np.int64)

    r0s = {}
    for reg, (lo, hi) in enumerate(g.reg_bounds):
        rs = hi - lo
        # rows are C+1 wide: col C holds the fused ones column for sv
        r0 = np.zeros((128, max(rs, 0) + 1, C + 1), BF)
        r0[:, :, C] = BF(1.0)
        if rs > 0:
            inreg = np.flatnonzero((slot_of >= lo) & (slot_of < hi))
            # --- colliding roots: balanced (partition, batch) assignment
            coll = inreg[counts[inreg] > 1]
            coll = coll[np.argsort(-counts[coll], kind="stable")]
            nb = np.zeros(128, np.int64)
            used = np.zeros((rs, 128), bool)
            exi[:, reg * NB : (reg + 1) * NB] = (
                np.arange(128)[:, None] * (rs + 1) + rs      # dump rows
            )
            for u in coll:
                srel = slot_of[u] - lo
                cand = np.argsort(nb, kind="stable")
                p = -1
                for c in cand:
                    if nb[c] < NB and slices[nb[c]] >= counts[u] and not used[srel, c]:
                        p = c
                        break
                assert p >= 0, f"fold capacity exceeded (reg {reg})"
                b = nb[p]
                nb[p] += 1
                used[srel, p] = True
                row_of[u] = p
                exi[p, reg * NB + b] = p * (rs + 1) + srel
                k0 = seg_start[u]
                for k in range(counts[u]):
                    exf[p, reg * g.npair + pair_base[b] + k] = feats_s[k0 + k]
            # --- singles: fill remaining rows per slot in cell order
            for srel in range(rs):
                s = lo + srel
                in_slot = inreg[slot_of[inreg] == s]
                sing = in_slot[counts[in_slot] == 1]
                free = np.flatnonzero(~used[srel])
                assert len(sing) <= len(free), f"slot overflow (slot {s})"
                row_of[sing] = free[: len(sing)]
                r0[free[: len(sing)], srel, :C] = feats_s[seg_start[sing]]
                cr = in_slot[counts[in_slot] > 1]
                r0[row_of[cr], srel, :C] = feats_s[seg_start[cr]]
        r0s[f"r0_{reg}"] = r0.reshape(128 * (max(rs, 0) + 1), C + 1)

    assert (row_of >= 0).all()
    sel[row_of, slot_of, jloc] = BF(1.0)
    out = {"exi": exi, "exf": exf,
           "selt": np.ascontiguousarray(sel.reshape(128, NS * SC))}
    out.update(r0s)
    return out


def prep_inputs(g: Geo, features, coordinates, conv_w, gamma, beta):
    feats = np.ascontiguousarray(features, np.float32)
    coords = np.asarray(coordinates)
    b, y, x = coords[:, 0], coords[:, 2], coords[:, 3]
    strip = y // g.ystrip
    wt = np.ascontiguousarray(np.asarray(conv_w).T.astype(BF))          # [C, O]
    gam = np.ascontiguousarray(
        np.asarray(gamma, np.float32).reshape(g.O // 128, 128).T)       # [128, OCH]
    bet = np.ascontiguousarray(
        np.asarray(beta, np.float32).reshape(g.O // 128, 128).T)
    in_maps = []
    for core in range(g.ncores):
        bb, st = divmod(core, g.NSTRIP)
        m = (b == bb) & (strip == st)
        cell = (y[m] - st * g.ystrip) * g.W + x[m]
        shard = prep_shard(g, feats[m], cell.astype(np.int64))
        shard.update({"wt": wt, "gamma": gam, "beta": bet})
        in_maps.append(shard)
    return in_maps


# --------------------------------------------------------------------------
# device program
# --------------------------------------------------------------------------

DEBUG_V = False
MULTI_IDX_SCATTER = False   # multi-index indirect DMA loses writes on HW


def build_program(g: Geo) -> bass.Bass:
    C, O = g.C, g.O
    OCH = O // 128
    NS, SC, NB = g.nslot, g.slot_cells, g.NB
    NQ = g.nquad
    QW = g.QUAD * SC
    slices = g.SLICES
    pair_base = [0]
    for s in slices[:-1]:
        pair_base.append(pair_base[-1] + s)

    nc = bacc.Bacc(num_devices=g.ncores)
    r0_d = [
        nc.declare_dram_parameter(
            f"r0_{r}", [128 * (max(hi - lo, 0) + 1), C + 1], BF16, False
        )
        for r, (lo, hi) in enumerate(g.reg_bounds)
    ]
    exi_d = nc.declare_dram_parameter("exi", [128, NB * g.NREG], I32, False)
    exf_d = nc.declare_dram_parameter("exf", [128, g.npair * g.NREG, C], BF16, False)
    selt_d = nc.declare_dram_parameter("selt", [128, NS * SC], BF16, False)
    wt_d = nc.declare_dram_parameter("wt", [C, O], BF16, False)
    gam_d = nc.declare_dram_parameter("gamma", [128, OCH], F32, False)
    bet_d = nc.declare_dram_parameter("beta", [128, OCH], F32, False)
    out_d = nc.declare_dram_parameter("out", [O, g.cells], BF16, True)
    cc_in = nc.dram_tensor("cc_in", [128, 2 * OCH], F32)
    cc_out = nc.dram_tensor("cc_out", [128, 2 * OCH], F32, addr_space="Shared")

    with tile.TileContext(nc) as tc:
        with (
            tc.tile_pool(name="vstore", bufs=1) as vstore,
            tc.tile_pool(name="singles", bufs=1) as singles,
            tc.tile_pool(name="exfp", bufs=2) as exfp,
            tc.tile_pool(name="gtfp", bufs=2) as gtfp,
            tc.tile_pool(name="selp", bufs=5) as selp,
            tc.tile_pool(name="gtq", bufs=g.nquad) as gtqp,
            tc.tile_pool(name="osb", bufs=4) as opool,
            tc.tile_pool(name="pstat", bufs=2, space="PSUM") as pstat,
            tc.tile_pool(name="pf", bufs=3, space="PSUM") as pf,
        ):
            # ---- small inputs
            wt_sb = singles.tile([C, O], BF16)
            nc.sync.dma_start(out=wt_sb[:], in_=wt_d[:, :])
            gam_sb = singles.tile([128, OCH], F32)
            nc.sync.dma_start(out=gam_sb[:], in_=gam_d[:, :])
            bet_sb = singles.tile([128, OCH], F32)
            nc.sync.dma_start(out=bet_sb[:], in_=bet_d[:, :])
            exi_sb = singles.tile([128, NB * g.NREG], I32)
            nc.sync.dma_start(out=exi_sb[:], in_=exi_d[:, :])
            ones_col = singles.tile([128, 1], F32)
            nc.vector.memset(ones_col[:], 1.0)
            ones_row = singles.tile([128, 128], F32)
            nc.vector.memset(ones_row[:], 1.0)
            zeros_row = singles.tile([128, 128], F32)
            nc.vector.memset(zeros_row[:], 0.0)
            ident = singles.tile([128, 128], F32)
            nc.gpsimd.affine_select(
                out=ident[:], in_=ones_row[:], pattern=[[1, 128]],
                compare_op=mybir.AluOpType.is_equal, fill=0.0,
                base=0, channel_multiplier=-1,
            )
            eps_t = singles.tile([128, 1], F32)
            nc.vector.memset(eps_t[:], float(g.EPS))

            v_all = vstore.tile([128, NS, C + 1], BF16)

            # ---- per-region: fold extras, scatter into r0 in place, load V
            for reg, (lo, hi) in enumerate(g.reg_bounds):
                rs = hi - lo
                if rs <= 0:
                    continue
                exf_t = exfp.tile([128, g.npair, C], BF16, tag="exf")
                nc.sync.dma_start(
                    out=exf_t[:],
                    in_=exf_d[:, reg * g.npair : (reg + 1) * g.npair, :],
                )
                gtf = gtfp.tile([128, NB, C + 1], BF16, tag="gtf")
                nc.vector.memset(gtf[:, :, C : C + 1], 1.0)
                for b in range(NB):
                    base = pair_base[b]
                    nc.vector.tensor_tensor(
                        out=gtf[:, b, :C], in0=exf_t[:, base, :],
                        in1=exf_t[:, base + 1, :], op=mybir.AluOpType.max,
                    )
                    for k in range(2, slices[b]):
                        nc.vector.tensor_tensor(
                            out=gtf[:, b, :C], in0=gtf[:, b, :C],
                            in1=exf_t[:, base + k, :], op=mybir.AluOpType.max,
                        )
                if MULTI_IDX_SCATTER:
                    nc.gpsimd.indirect_dma_start(
                        out=r0_d[reg][:, :],
                        out_offset=bass.IndirectOffsetOnAxis(
                            ap=exi_sb[:, reg * NB : (reg + 1) * NB], axis=0
                        ),
                        in_=gtf[:, :, :], in_offset=None,
                    )
                else:
                    for b in range(NB):
                        nc.gpsimd.indirect_dma_start(
                            out=r0_d[reg][:, :],
                            out_offset=bass.IndirectOffsetOnAxis(
                                ap=exi_sb[:, reg * NB + b : reg * NB + b + 1], axis=0
                            ),
                            in_=gtf[:, b, :], in_offset=None,
                        )
                r3 = r0_d[reg].ap().rearrange("(p s) c -> p s c", s=rs + 1)
                nc.gpsimd.dma_start(
                    out=v_all[:, lo:hi, :], in_=r3[:, :rs, :]
                )

            # ---- Sigma = sum_s V_s^T [V_s | 1]  (bf16, PSUM f32)
            sig_ps = pstat.tile([128, C + 1], F32, space="PSUM", tag="st")
            for s in range(NS):
                nc.tensor.matmul(
                    out=sig_ps[:],
                    lhsT=v_all[:, s, :C],
                    rhs=v_all[:, s, :],
                    start=(s == 0), stop=(s == NS - 1),
                )
            sig_bf = singles.tile([128, C + 1], BF16)
            nc.vector.tensor_copy(out=sig_bf[:], in_=sig_ps[:])
            if DEBUG_V:
                dbg_v = nc.declare_dram_parameter(
                    "dbg_v", [128, NS, C + 1], BF16, True
                )
                nc.sync.dma_start(out=dbg_v[:, :, :], in_=v_all[:, :, :])

            # ---- project stats to per-channel sums (pre-collective)
            a_ps = pstat.tile([128, O], F32, space="PSUM", tag="st")
            nc.tensor.matmul(
                out=a_ps[:], lhsT=sig_bf[:, :C], rhs=wt_sb[:],
                start=True, stop=True,
            )
            bsb = singles.tile([128, O], F32)
            nc.vector.tensor_tensor(
                out=bsb[:], in0=a_ps[:], in1=wt_sb[:], op=mybir.AluOpType.mult
            )
            red_ps = pstat.tile([128, 2 * OCH], F32, space="PSUM", tag="st")
            for ch in range(OCH):
                nc.tensor.matmul(
                    out=red_ps[:, ch : ch + 1],
                    lhsT=bsb[:, ch * 128 : (ch + 1) * 128],
                    rhs=ones_col[:], start=True, stop=True,
                )
                nc.tensor.matmul(
                    out=red_ps[:, OCH + ch : OCH + ch + 1],
                    lhsT=wt_sb[:, ch * 128 : (ch + 1) * 128],
                    rhs=sig_bf[:, C : C + 1], start=True, stop=True,
                )
            red_loc = singles.tile([128, 2 * OCH], F32)
            nc.vector.tensor_copy(out=red_loc[:], in_=red_ps[:])
            # dispatched from ACT so the sync engine's sel prefetch stream
            # is not blocked behind the sigma wait
            nc.scalar.dma_start(out=cc_in[:, :], in_=red_loc[:])
            nc.gpsimd.collective_compute(
                "AllReduce",
                mybir.AluOpType.add,
                replica_groups=[list(range(g.ncores))],
                ins=[cc_in.ap().opt()],
                outs=[cc_out.ap().opt()],
            )

            # ---- phase C front: sel loads (2 quads per DMA), GT matmuls,
            # PSUM->bf16 SBUF copies alternating DVE/ACT. All fronts are
            # emitted before the collective is consumed, so they cover it.
            gtq_tiles = {}

            def front(q):
                s0 = q * g.QUAD
                sq = min(g.QUAD, NS - s0)
                w = min(QW, g.cells - s0 * SC)
                if q % 2 == 0:
                    w2 = min(2 * QW, g.cells - s0 * SC)
                    sel_sb = selp.tile([128, 2 * QW], BF16, tag="sel",
                                       name=f"sel{(q // 2) % 5}")
                    nc.sync.dma_start(
                        out=sel_sb[:, :w2], in_=selt_d[:, s0 * SC : s0 * SC + w2]
                    )
                    front.sel = sel_sb
                    off = 0
                else:
                    sel_sb = front.sel
                    off = QW
                gt_ps = pf.tile([128, QW], F32, space="PSUM", tag="fp")
                for i in range(sq):
                    n_s = min(SC, g.cells - (s0 + i) * SC)
                    nc.tensor.matmul(
                        out=gt_ps[:, i * SC : i * SC + n_s],
                        lhsT=v_all[:, s0 + i, :C],
                        rhs=sel_sb[:, off + i * SC : off + i * SC + n_s],
                        start=True, stop=True,
                    )
                gq = gtqp.tile([128, QW], BF16, tag="gtq", name=f"gq{q}")
                if q % 2 == 0:
                    nc.vector.tensor_copy(out=gq[:, :w], in_=gt_ps[:, :w])
                else:
                    nc.scalar.copy(out=gq[:, :w], in_=gt_ps[:, :w])
                gtq_tiles[q] = (gq, w)

            for q in range(NQ):
                front(q)

            # ---- post-collective BN constants
            mom = singles.tile([128, 2 * OCH], F32)
            nc.sync.dma_start(out=mom[:], in_=cc_out[:, :])
            nc.scalar.mul(out=mom[:], in_=mom[:], mul=1.0 / float(g.ncell_total))
            var_t = singles.tile([128, OCH], F32)
            nc.vector.tensor_tensor(
                out=var_t[:], in0=mom[:, OCH:], in1=mom[:, OCH:],
                op=mybir.AluOpType.mult,
            )
            nc.vector.tensor_tensor(
                out=var_t[:], in0=mom[:, :OCH], in1=var_t[:],
                op=mybir.AluOpType.subtract,
            )
            rstd = singles.tile([128, OCH], F32)
            nc.scalar.activation(
                out=rstd[:], in_=var_t[:],
                func=mybir.ActivationFunctionType.Sqrt, bias=eps_t[:],
            )
            nc.vector.reciprocal(out=rstd[:], in_=rstd[:])
            # a_t64: channel ch's scale vector in column ch*32 (so its
            # transposed row lands on partition ch*32 for the broadcast matmul)
            a_t64 = singles.tile([128, OCH * 32], F32)
            b_t = singles.tile([128, OCH], F32)
            for ch in range(OCH):
                nc.vector.tensor_tensor(
                    out=a_t64[:, ch * 32 : ch * 32 + 1],
                    in0=gam_sb[:, ch : ch + 1], in1=rstd[:, ch : ch + 1],
                    op=mybir.AluOpType.mult,
                )
                nc.vector.tensor_tensor(
                    out=b_t[:, ch : ch + 1],
                    in0=mom[:, OCH + ch : OCH + ch + 1],
                    in1=a_t64[:, ch * 32 : ch * 32 + 1],
                    op=mybir.AluOpType.mult,
                )
                nc.vector.tensor_tensor(
                    out=b_t[:, ch : ch + 1],
                    in0=bet_sb[:, ch : ch + 1], in1=b_t[:, ch : ch + 1],
                    op=mybir.AluOpType.subtract,
                )
            # a2[p, ch*128+j] = a(ch, j): replicate a-column along free axis
            # (DVE), then matmul with the identity to transpose the broadcast.
            a_rep = singles.tile([128, O], F32)
            for ch in range(OCH):
                nc.vector.tensor_scalar(
                    out=a_rep[:, ch * 128 : (ch + 1) * 128],
                    in0=zeros_row[:],
                    scalar1=a_t64[:, ch * 32 : ch * 32 + 1], scalar2=None,
                    op0=mybir.AluOpType.add,
                )
            a2_ps = pstat.tile([128, O], F32, space="PSUM", tag="st")
            for ch in range(OCH):
                nc.tensor.matmul(
                    out=a2_ps[:, ch * 128 : (ch + 1) * 128],
                    lhsT=a_rep[:, ch * 128 : (ch + 1) * 128],
                    rhs=ident[:],
                    start=True, stop=True,
                )
            wt_bn = singles.tile([C, O], BF16)
            nc.vector.tensor_tensor(
                out=wt_bn[:], in0=wt_sb[:], in1=a2_ps[:], op=mybir.AluOpType.mult
            )
            if DEBUG_V:
                a2_sb = singles.tile([128, O], F32)
                nc.vector.tensor_copy(out=a2_sb[:], in_=a2_ps[:])
                dbg_a2 = nc.declare_dram_parameter("dbg_a2", [128, O], F32, True)
                nc.sync.dma_start(out=dbg_a2[:, :], in_=a2_sb[:])
                dbg_bt = nc.declare_dram_parameter("dbg_bt", [128, OCH], F32, True)
                nc.sync.dma_start(out=dbg_bt[:, :], in_=b_t[:])
                dbg_wb = nc.declare_dram_parameter("dbg_wb", [C, O], BF16, True)
                nc.sync.dma_start(out=dbg_wb[:, :], in_=wt_bn[:])
                dbg_id = nc.declare_dram_parameter("dbg_id", [128, 128], F32, True)
                nc.sync.dma_start(out=dbg_id[:, :], in_=ident[:])

            # ---- phase C back: conv, relu(x+b) alternating ACT/DVE, store
            # two quads per DMA (4 KB per-partition descriptors).
            ot_pair = {}

            def back(q):
                gq, w = gtq_tiles.pop(q)
                first = q % 2 == 0
                off = 0 if first else QW
                for ch in range(OCH):
                    fp = pf.tile([128, QW], F32, space="PSUM", tag="fp")
                    for h0 in range(0, w, 512):   # PSUM-bank-limited N<=512
                        hw_ = min(512, w - h0)
                        nc.tensor.matmul(
                            out=fp[:, h0 : h0 + hw_],
                            lhsT=wt_bn[:, ch * 128 : (ch + 1) * 128],
                            rhs=gq[:, h0 : h0 + hw_],
                            start=True, stop=True,
                        )
                    if first:
                        ot_pair[ch] = opool.tile(
                            [128, 2 * QW], BF16, tag="ot", name=f"ot{ch}{(q // 2) % 2}"
                        )
                    ot = ot_pair[ch]
                    if (2 * q + ch) % 16 < 9:   # ACT is ~1.15x faster per tile
                        nc.scalar.activation(
                            out=ot[:, off : off + w], in_=fp[:, :w],
                            func=mybir.ActivationFunctionType.Relu,
                            bias=b_t[:, ch : ch + 1],
                        )
                    else:
                        nc.vector.tensor_scalar(
                            out=ot[:, off : off + w], in0=fp[:, :w],
                            scalar1=b_t[:, ch : ch + 1], scalar2=0.0,
                            op0=mybir.AluOpType.add, op1=mybir.AluOpType.max,
                        )
                    if (not first) or q == NQ - 1:
                        ww = off + w
                        base = (q - (0 if first else 1)) * QW
                        nc.sync.dma_start(
                            out=out_d[ch * 128 : (ch + 1) * 128, base : base + ww],
                            in_=ot[:, :ww],
                        )

            for q in range(NQ):
                back(q)
    return nc


_PROGRAM_CACHE: dict = {}


def get_program(g: Geo) -> bass.Bass:
    if g not in _PROGRAM_CACHE:
        nc = build_program(g)
        nc.finalize()
        _PROGRAM_CACHE[g] = nc
    return _PROGRAM_CACHE[g]


def assemble_output(g: Geo, per_core: list) -> np.ndarray:
    out = np.empty((g.B, g.O, g.H, g.W), np.float32)
    for core in range(g.ncores):
        bb, st = divmod(core, g.NSTRIP)
        out[bb, :, st * g.ystrip : (st + 1) * g.ystrip, :] = (
            np.asarray(per_core[core]).astype(np.float32).reshape(g.O, g.ystrip, g.W)
        )
    return out


def kernel(features, coordinates, conv_w, gamma, beta):
    g = GEO
    in_maps = prep_inputs(g, features, coordinates, conv_w, gamma, beta)
    nc = get_program(g)
    res = run_bass_kernel_spmd(nc, in_maps, core_ids=list(range(g.ncores)))
    return assemble_output(g, [r["out"] for r in res.results])


# revision 36
# speedup vs baseline: 1.0033x; 1.0033x over previous
"""BEV feature extractor (scatter-max -> 1x1 conv -> BN(train) -> ReLU) on 8 TRN2 cores.

Sharding: data-parallel over (batch, y-strip) -> 8 shards, BN stats all-reduced.

bf16 data path (tolerance 2e-2; bf16 rounding ~4e-3 and BN partially cancels it):

  1. Host packs each shard partition-major: region tensors r0_r [128, RS+1, C]
     hold root features of slot s at [row, s-lo, :] (row = cell's row in the
     slot's 128; last column = dump rows). Colliding cells' points (root +
     extras) are packed into exf fold batches, one batch row per partition,
     balanced so each (region, partition) needs at most NB batches.
  2. Device folds: DVE max-chains over exf slices -> gtf [128, NB, C]; one
     multi-index indirect scatter per region writes the folded rows back into
     r0_r in place (unused rows land on per-partition dump rows). Then the
     region slab DMAs into the SBUF-resident V tile v_all [128, NS, C+1]
     (ones column fused for the sv reduction).
  3. PE accumulates Sigma = sum_s V_s^T [V_s | 1] (bf16). BN stats project to
     per-channel sums BEFORE the collective: ex2_o = w_o^T Sigma w_o,
     mn_o = w_o^T sv, so the AllReduce payload is [128, 2*OCH] (2 KB).
  4. Phase C per quad (4 slots = 1024 cells): GT = V_s^T Sel_s (Sel is a
     host-built bf16 0/1 matrix streamed from DRAM), conv rhs is the bf16 GT
     quad, conv weights are pre-scaled by the BN 'a' (folded on device), so
     the epilogue is relu(x + b) - a single pass alternating ACT / DVE.
     Output is stored bf16 ([O, cells]) and upcast on host.
  5. The collective overlaps with GT run-ahead for the first RA quads.
"""

import math
from dataclasses import dataclass

import ml_dtypes
import numpy as np

import concourse.bass as bass
import concourse.tile as tile
from concourse import bacc, mybir
from concourse.bass_utils import run_bass_kernel_spmd

F32 = mybir.dt.float32
BF16 = mybir.dt.bfloat16
I32 = mybir.dt.int32
BF = ml_dtypes.bfloat16


@dataclass(frozen=True)
class Geo:
    B: int = 2
    H: int = 400
    W: int = 400
    C: int = 128            # input channels (= partition count)
    O: int = 256            # output channels (multiple of 128)
    NSTRIP: int = 4         # y-strips per batch; B*NSTRIP = 8 cores
    SLOT_BLKS: int = 2      # 128-cell blocks packed per 128-row slot
    NREG: int = 4           # independent fold/load regions
    SLICES: tuple = (7, 3, 2, 2, 2)      # points (root+extras) per fold batch
    QUAD: int = 4           # slots per phase-C tile
    RA: int = 16            # run-ahead quads across the collective
    EPS: float = 1e-5

    @property
    def ystrip(self):
        return self.H // self.NSTRIP

    @property
    def cells(self):
        return self.ystrip * self.W

    @property
    def ncores(self):
        return self.B * self.NSTRIP

    @property
    def slot_cells(self):
        return 128 * self.SLOT_BLKS

    @property
    def nslot(self):
        return math.ceil(self.cells / self.slot_cells)

    @property
    def NB(self):
        return len(self.SLICES)

    @property
    def npair(self):                 # exf slices per region
        return sum(self.SLICES)

    @property
    def nquad(self):
        return math.ceil(self.nslot / self.QUAD)

    @property
    def ncell_total(self):
        return self.B * self.H * self.W

    @property
    def reg_bounds(self):
        rs = math.ceil(self.nslot / self.NREG)
        out = []
        for reg in range(self.NREG):
            lo = min(reg * rs, self.nslot)
            hi = self.nslot if reg == self.NREG - 1 else min((reg + 1) * rs, self.nslot)
            out.append((lo, hi))
        return out


GEO = Geo()


# --------------------------------------------------------------------------
# host-side shard prep
# --------------------------------------------------------------------------

def prep_shard(g: Geo, feats: np.ndarray, cell: np.ndarray) -> dict:
    """feats [n, C] f32, cell [n] int in [0, g.cells)."""
    C, SC, NS, NB = g.C, g.slot_cells, g.nslot, g.NB
    slices = g.SLICES
    pair_base = np.cumsum((0,) + slices[:-1])

    order = np.argsort(cell, kind="stable")
    cell_s = cell[order]
    feats_s = feats[order].astype(BF)
    uniq, seg_start, counts = np.unique(
        cell_s, return_index=True, return_counts=True
    )
    slot_of = uniq // SC
    jloc = uniq % SC

    exi = np.zeros((128, NB * g.NREG), np.int32)
    exf = np.zeros((128, g.npair * g.NREG, C), BF)
    sel = np.zeros((128, NS, SC), BF)
    row_of = np.full(len(uniq), -1, np.int64)

    r0s = {}
    for reg, (lo, hi) in enumerate(g.reg_bounds):
        rs = hi - lo
        # rows are C+1 wide: col C holds the fused ones column for sv
        r0 = np.zeros((128, max(rs, 0) + 1, C + 1), BF)
        r0[:, :, C] = BF(1.0)
        if rs > 0:
            inreg = np.flatnonzero((slot_of >= lo) & (slot_of < hi))
            # --- colliding roots: balanced (partition, batch) assignment
            coll = inreg[counts[inreg] > 1]
            coll = coll[np.argsort(-counts[coll], kind="stable")]
            nb = np.zeros(128, np.int64)
            used = np.zeros((rs, 128), bool)
            exi[:, reg * NB : (reg + 1) * NB] = (
                np.arange(128)[:, None] * (rs + 1) + rs      # dump rows
            )
            for u in coll:
                srel = slot_of[u] - lo
                cand = np.argsort(nb, kind="stable")
                p = -1
                for c in cand:
                    if nb[c] < NB and slices[nb[c]] >= counts[u] and not used[srel, c]:
                        p = c
                        break
                assert p >= 0, f"fold capacity exceeded (reg {reg})"
                b = nb[p]
                nb[p] += 1
                used[srel, p] = True
                row_of[u] = p
                exi[p, reg * NB + b] = p * (rs + 1) + srel
                k0 = seg_start[u]
                for k in range(counts[u]):
                    exf[p, reg * g.npair + pair_base[b] + k] = feats_s[k0 + k]
            # --- singles: fill remaining rows per slot in cell order
            for srel in range(rs):
                s = lo + srel
                in_slot = inreg[slot_of[inreg] == s]
                sing = in_slot[counts[in_slot] == 1]
                free = np.flatnonzero(~used[srel])
                assert len(sing) <= len(free), f"slot overflow (slot {s})"
                row_of[sing] = free[: len(sing)]
                r0[free[: len(sing)], srel, :C] = feats_s[seg_start[sing]]
                cr = in_slot[counts[in_slot] > 1]
                r0[row_of[cr], srel, :C] = feats_s[seg_start[cr]]
        r0s[f"r0_{reg}"] = r0.reshape(128 * (max(rs, 0) + 1), C + 1)

    assert (row_of >= 0).all()
    sel[row_of, slot_of, jloc] = BF(1.0)
    out = {"exi": exi, "exf": exf,
           "selt": np.ascontiguousarray(sel.reshape(128, NS * SC))}
    out.update(r0s)
    return out


def prep_inputs(g: Geo, features, coordinates, conv_w, gamma, beta):
    feats = np.ascontiguousarray(features, np.float32)
    coords = np.asarray(coordinates)
    b, y, x = coords[:, 0], coords[:, 2], coords[:, 3]
    strip = y // g.ystrip
    wt = np.ascontiguousarray(np.asarray(conv_w).T.astype(BF))          # [C, O]
    gam = np.ascontiguousarray(
        np.asarray(gamma, np.float32).reshape(g.O // 128, 128).T)       # [128, OCH]
    bet = np.ascontiguousarray(
        np.asarray(beta, np.float32).reshape(g.O // 128, 128).T)
    in_maps = []
    for core in range(g.ncores):
        bb, st = divmod(core, g.NSTRIP)
        m = (b == bb) & (strip == st)
        cell = (y[m] - st * g.ystrip) * g.W + x[m]
        shard = prep_shard(g, feats[m], cell.astype(np.int64))
        shard.update({"wt": wt, "gamma": gam, "beta": bet})
        in_maps.append(shard)
    return in_maps


# --------------------------------------------------------------------------
# device program
# --------------------------------------------------------------------------

DEBUG_V = False
MULTI_IDX_SCATTER = False   # multi-index indirect DMA loses writes on HW


def build_program(g: Geo) -> bass.Bass:
    C, O = g.C, g.O
    OCH = O // 128
    NS, SC, NB = g.nslot, g.slot_cells, g.NB
    NQ = g.nquad
    QW = g.QUAD * SC
    slices = g.SLICES
    pair_base = [0]
    for s in slices[:-1]:
        pair_base.append(pair_base[-1] + s)

    nc = bacc.Bacc(num_devices=g.ncores)
    r0_d = [
        nc.declare_dram_parameter(
            f"r0_{r}", [128 * (max(hi - lo, 0) + 1), C + 1], BF16, False
        )
        for r, (lo, hi) in enumerate(g.reg_bounds)
    ]
    exi_d = nc.declare_dram_parameter("exi", [128, NB * g.NREG], I32, False)
    exf_d = nc.declare_dram_parameter("exf", [128, g.npair * g.NREG, C], BF16, False)
    selt_d = nc.declare_dram_parameter("selt", [128, NS * SC], BF16, False)
    wt_d = nc.declare_dram_parameter("wt", [C, O], BF16, False)
    gam_d = nc.declare_dram_parameter("gamma", [128, OCH], F32, False)
    bet_d = nc.declare_dram_parameter("beta", [128, OCH], F32, False)
    out_d = nc.declare_dram_parameter("out", [O, g.cells], BF16, True)
    cc_in = nc.dram_tensor("cc_in", [128, 2 * OCH], F32)
    cc_out = nc.dram_tensor("cc_out", [128, 2 * OCH], F32, addr_space="Shared")

    with tile.TileContext(nc) as tc:
        with (
            tc.tile_pool(name="vstore", bufs=1) as vstore,
            tc.tile_pool(name="singles", bufs=1) as singles,
            tc.tile_pool(name="exfp", bufs=2) as exfp,
            tc.tile_pool(name="gtfp", bufs=2) as gtfp,
            tc.tile_pool(name="selp", bufs=3) as selp,
            tc.tile_pool(name="gtq", bufs=g.nquad) as gtqp,
            tc.tile_pool(name="osb", bufs=4) as opool,
            tc.tile_pool(name="pstat", bufs=2, space="PSUM") as pstat,
            tc.tile_pool(name="pf", bufs=3, space="PSUM") as pf,
        ):
            # ---- small inputs
            wt_sb = singles.tile([C, O], BF16)
            nc.sync.dma_start(out=wt_sb[:], in_=wt_d[:, :])
            gam_sb = singles.tile([128, OCH], F32)
            nc.sync.dma_start(out=gam_sb[:], in_=gam_d[:, :])
            bet_sb = singles.tile([128, OCH], F32)
            nc.sync.dma_start(out=bet_sb[:], in_=bet_d[:, :])
            exi_sb = singles.tile([128, NB * g.NREG], I32)
            nc.sync.dma_start(out=exi_sb[:], in_=exi_d[:, :])
            ones_col = singles.tile([128, 1], F32)
            nc.vector.memset(ones_col[:], 1.0)
            ones_row = singles.tile([128, 128], F32)
            nc.vector.memset(ones_row[:], 1.0)
            zeros_row = singles.tile([128, 128], F32)
            nc.vector.memset(zeros_row[:], 0.0)
            ident = singles.tile([128, 128], F32)
            nc.gpsimd.affine_select(
                out=ident[:], in_=ones_row[:], pattern=[[1, 128]],
                compare_op=mybir.AluOpType.is_equal, fill=0.0,
                base=0, channel_multiplier=-1,
            )
            eps_t = singles.tile([128, 1], F32)
            nc.vector.memset(eps_t[:], float(g.EPS))

            v_all = vstore.tile([128, NS, C + 1], BF16)

            # ---- per-region: fold extras, scatter into r0 in place, load V
            for reg, (lo, hi) in enumerate(g.reg_bounds):
                rs = hi - lo
                if rs <= 0:
                    continue
                exf_t = exfp.tile([128, g.npair, C], BF16, tag="exf")
                nc.sync.dma_start(
                    out=exf_t[:],
                    in_=exf_d[:, reg * g.npair : (reg + 1) * g.npair, :],
                )
                gtf = gtfp.tile([128, NB, C + 1], BF16, tag="gtf")
                nc.vector.memset(gtf[:, :, C : C + 1], 1.0)
                for b in range(NB):
                    base = pair_base[b]
                    nc.vector.tensor_tensor(
                        out=gtf[:, b, :C], in0=exf_t[:, base, :],
                        in1=exf_t[:, base + 1, :], op=mybir.AluOpType.max,
                    )
                    for k in range(2, slices[b]):
                        nc.vector.tensor_tensor(
                            out=gtf[:, b, :C], in0=gtf[:, b, :C],
                            in1=exf_t[:, base + k, :], op=mybir.AluOpType.max,
                        )
                if MULTI_IDX_SCATTER:
                    nc.gpsimd.indirect_dma_start(
                        out=r0_d[reg][:, :],
                        out_offset=bass.IndirectOffsetOnAxis(
                            ap=exi_sb[:, reg * NB : (reg + 1) * NB], axis=0
                        ),
                        in_=gtf[:, :, :], in_offset=None,
                    )
                else:
                    for b in range(NB):
                        nc.gpsimd.indirect_dma_start(
                            out=r0_d[reg][:, :],
                            out_offset=bass.IndirectOffsetOnAxis(
                                ap=exi_sb[:, reg * NB + b : reg * NB + b + 1], axis=0
                            ),
                            in_=gtf[:, b, :], in_offset=None,
                        )
                r3 = r0_d[reg].ap().rearrange("(p s) c -> p s c", s=rs + 1)
                nc.gpsimd.dma_start(
                    out=v_all[:, lo:hi, :], in_=r3[:, :rs, :]
                )

            # ---- Sigma = sum_s V_s^T [V_s | 1]  (bf16, PSUM f32)
            sig_ps = pstat.tile([128, C + 1], F32, space="PSUM", tag="st")
            for s in range(NS):
                nc.tensor.matmul(
                    out=sig_ps[:],
                    lhsT=v_all[:, s, :C],
                    rhs=v_all[:, s, :],
                    start=(s == 0), stop=(s == NS - 1),
                )
            sig_bf = singles.tile([128, C + 1], BF16)
            nc.vector.tensor_copy(out=sig_bf[:], in_=sig_ps[:])
            if DEBUG_V:
                dbg_v = nc.declare_dram_parameter(
                    "dbg_v", [128, NS, C + 1], BF16, True
                )
                nc.sync.dma_start(out=dbg_v[:, :, :], in_=v_all[:, :, :])

            # ---- project stats to per-channel sums (pre-collective)
            a_ps = pstat.tile([128, O], F32, space="PSUM", tag="st")
            nc.tensor.matmul(
                out=a_ps[:], lhsT=sig_bf[:, :C], rhs=wt_sb[:],
                start=True, stop=True,
            )
            bsb = singles.tile([128, O], F32)
            nc.vector.tensor_tensor(
                out=bsb[:], in0=a_ps[:], in1=wt_sb[:], op=mybir.AluOpType.mult
            )
            red_ps = pstat.tile([128, 2 * OCH], F32, space="PSUM", tag="st")
            for ch in range(OCH):
                nc.tensor.matmul(
                    out=red_ps[:, ch : ch + 1],
                    lhsT=bsb[:, ch * 128 : (ch + 1) * 128],
                    rhs=ones_col[:], start=True, stop=True,
                )
                nc.tensor.matmul(
                    out=red_ps[:, OCH + ch : OCH + ch + 1],
                    lhsT=wt_sb[:, ch * 128 : (ch + 1) * 128],
                    rhs=sig_bf[:, C : C + 1], start=True, stop=True,
                )
            red_loc = singles.tile([128, 2 * OCH], F32)
            nc.vector.tensor_copy(out=red_loc[:], in_=red_ps[:])
            # dispatched from ACT so the sync engine's sel prefetch stream
            # is not blocked behind the sigma wait
            nc.scalar.dma_start(out=cc_in[:, :], in_=red_loc[:])
            nc.gpsimd.collective_compute(
                "AllReduce",
                mybir.AluOpType.add,
                replica_groups=[list(range(g.ncores))],
                ins=[cc_in.ap().opt()],
                outs=[cc_out.ap().opt()],
            )

            # ---- phase C front: sel loads (2 quads per DMA), GT matmuls,
            # PSUM->bf16 SBUF copies alternating DVE/ACT. All fronts are
            # emitted before the collective is consumed, so they cover it.
            gtq_tiles = {}

            def front(q):
                s0 = q * g.QUAD
                sq = min(g.QUAD, NS - s0)
                w = min(QW, g.cells - s0 * SC)
                if q % 2 == 0:
                    w2 = min(2 * QW, g.cells - s0 * SC)
                    sel_sb = selp.tile([128, 2 * QW], BF16, tag="sel",
                                       name=f"sel{(q // 2) % 3}")
                    nc.sync.dma_start(
                        out=sel_sb[:, :w2], in_=selt_d[:, s0 * SC : s0 * SC + w2]
                    )
                    front.sel = sel_sb
                    off = 0
                else:
                    sel_sb = front.sel
                    off = QW
                gt_ps = pf.tile([128, QW], F32, space="PSUM", tag="fp")
                for i in range(sq):
                    n_s = min(SC, g.cells - (s0 + i) * SC)
                    nc.tensor.matmul(
                        out=gt_ps[:, i * SC : i * SC + n_s],
                        lhsT=v_all[:, s0 + i, :C],
                        rhs=sel_sb[:, off + i * SC : off + i * SC + n_s],
                        start=True, stop=True,
                    )
                gq = gtqp.tile([128, QW], BF16, tag="gtq", name=f"gq{q}")
                if q % 2 == 0:
                    nc.vector.tensor_copy(out=gq[:, :w], in_=gt_ps[:, :w])
                else:
                    nc.scalar.copy(out=gq[:, :w], in_=gt_ps[:, :w])
                gtq_tiles[q] = (gq, w)

            for q in range(NQ):
                front(q)

            # ---- post-collective BN constants
            mom = singles.tile([128, 2 * OCH], F32)
            nc.sync.dma_start(out=mom[:], in_=cc_out[:, :])
            nc.scalar.mul(out=mom[:], in_=mom[:], mul=1.0 / float(g.ncell_total))
            var_t = singles.tile([128, OCH], F32)
            nc.vector.tensor_tensor(
                out=var_t[:], in0=mom[:, OCH:], in1=mom[:, OCH:],
                op=mybir.AluOpType.mult,
            )
            nc.vector.tensor_tensor(
                out=var_t[:], in0=mom[:, :OCH], in1=var_t[:],
                op=mybir.AluOpType.subtract,
            )
            rstd = singles.tile([128, OCH], F32)
            nc.scalar.activation(
                out=rstd[:], in_=var_t[:],
                func=mybir.ActivationFunctionType.Sqrt, bias=eps_t[:],
            )
            nc.vector.reciprocal(out=rstd[:], in_=rstd[:])
            # a_t64: channel ch's scale vector in column ch*32 (so its
            # transposed row lands on partition ch*32 for the broadcast matmul)
            a_t64 = singles.tile([128, OCH * 32], F32)
            b_t = singles.tile([128, OCH], F32)
            for ch in range(OCH):
                nc.vector.tensor_tensor(
                    out=a_t64[:, ch * 32 : ch * 32 + 1],
                    in0=gam_sb[:, ch : ch + 1], in1=rstd[:, ch : ch + 1],
                    op=mybir.AluOpType.mult,
                )
                nc.vector.tensor_tensor(
                    out=b_t[:, ch : ch + 1],
                    in0=mom[:, OCH + ch : OCH + ch + 1],
                    in1=a_t64[:, ch * 32 : ch * 32 + 1],
                    op=mybir.AluOpType.mult,
                )
                nc.vector.tensor_tensor(
                    out=b_t[:, ch : ch + 1],
                    in0=bet_sb[:, ch : ch + 1], in1=b_t[:, ch : ch + 1],
                    op=mybir.AluOpType.subtract,
                )
            # a2[p, ch*128+j] = a(ch, j): replicate a-column along free axis
            # (DVE), then matmul with the identity to transpose the broadcast.
            a_rep = singles.tile([128, O], F32)
            for ch in range(OCH):
                nc.vector.tensor_scalar(
                    out=a_rep[:, ch * 128 : (ch + 1) * 128],
                    in0=zeros_row[:],
                    scalar1=a_t64[:, ch * 32 : ch * 32 + 1], scalar2=None,
                    op0=mybir.AluOpType.add,
                )
            a2_ps = pstat.tile([128, O], F32, space="PSUM", tag="st")
            for ch in range(OCH):
                nc.tensor.matmul(
                    out=a2_ps[:, ch * 128 : (ch + 1) * 128],
                    lhsT=a_rep[:, ch * 128 : (ch + 1) * 128],
                    rhs=ident[:],
                    start=True, stop=True,
                )
            wt_bn = singles.tile([C, O], BF16)
            nc.vector.tensor_tensor(
                out=wt_bn[:], in0=wt_sb[:], in1=a2_ps[:], op=mybir.AluOpType.mult
            )
            if DEBUG_V:
                a2_sb = singles.tile([128, O], F32)
                nc.vector.tensor_copy(out=a2_sb[:], in_=a2_ps[:])
                dbg_a2 = nc.declare_dram_parameter("dbg_a2", [128, O], F32, True)
                nc.sync.dma_start(out=dbg_a2[:, :], in_=a2_sb[:])
                dbg_bt = nc.declare_dram_parameter("dbg_bt", [128, OCH], F32, True)
                nc.sync.dma_start(out=dbg_bt[:, :], in_=b_t[:])
                dbg_wb = nc.declare_dram_parameter("dbg_wb", [C, O], BF16, True)
                nc.sync.dma_start(out=dbg_wb[:, :], in_=wt_bn[:])
                dbg_id = nc.declare_dram_parameter("dbg_id", [128, 128], F32, True)
                nc.sync.dma_start(out=dbg_id[:, :], in_=ident[:])

            # ---- phase C back: conv, relu(x+b) alternating ACT/DVE, store
            # two quads per DMA (4 KB per-partition descriptors).
            ot_pair = {}

            def back(q):
                gq, w = gtq_tiles.pop(q)
                first = q % 2 == 0
                off = 0 if first else QW
                for ch in range(OCH):
                    fp = pf.tile([128, QW], F32, space="PSUM", tag="fp")
                    for h0 in range(0, w, 512):   # PSUM-bank-limited N<=512
                        hw_ = min(512, w - h0)
                        nc.tensor.matmul(
                            out=fp[:, h0 : h0 + hw_],
                            lhsT=wt_bn[:, ch * 128 : (ch + 1) * 128],
                            rhs=gq[:, h0 : h0 + hw_],
                            start=True, stop=True,
                        )
                    if first:
                        ot_pair[ch] = opool.tile(
                            [128, 2 * QW], BF16, tag="ot", name=f"ot{ch}{(q // 2) % 2}"
                        )
                    ot = ot_pair[ch]
                    if (2 * q + ch) % 16 < 9:   # ACT is ~1.15x faster per tile
                        nc.scalar.activation(
                            out=ot[:, off : off + w], in_=fp[:, :w],
                            func=mybir.ActivationFunctionType.Relu,
                            bias=b_t[:, ch : ch + 1],
                        )
                    else:
                        nc.vector.tensor_scalar(
                            out=ot[:, off : off + w], in0=fp[:, :w],
                            scalar1=b_t[:, ch : ch + 1], scalar2=0.0,
                            op0=mybir.AluOpType.add, op1=mybir.AluOpType.max,
                        )
                    if (not first) or q == NQ - 1:
                        ww = off + w
                        base = (q - (0 if first else 1)) * QW
                        eng = nc.sync if (q // 2) % 2 == 0 else nc.gpsimd
                        eng.dma_start(
                            out=out_d[ch * 128 : (ch + 1) * 128, base : base + ww],
                            in_=ot[:, :ww],
                        )

            for q in range(NQ):
                back(q)
    return nc


_PROGRAM_CACHE: dict = {}


def get_program(g: Geo) -> bass.Bass:
    if g not in _PROGRAM_CACHE:
        nc = build_program(g)
        nc.finalize()
        _PROGRAM_CACHE[g] = nc
    return _PROGRAM_CACHE[g]


def assemble_output(g: Geo, per_core: list) -> np.ndarray:
    out = np.empty((g.B, g.O, g.H, g.W), np.float32)
    for core in range(g.ncores):
        bb, st = divmod(core, g.NSTRIP)
        out[bb, :, st * g.ystrip : (st + 1) * g.ystrip, :] = (
            np.asarray(per_core[core]).astype(np.float32).reshape(g.O, g.ystrip, g.W)
        )
    return out


def kernel(features, coordinates, conv_w, gamma, beta):
    g = GEO
    in_maps = prep_inputs(g, features, coordinates, conv_w, gamma, beta)
    nc = get_program(g)
    res = run_bass_kernel_spmd(nc, in_maps, core_ids=list(range(g.ncores)))
    return assemble_output(g, [r["out"] for r in res.results])
